# revision 15
# baseline (speedup 1.0000x reference)
"""TRN2 Bass kernel for nn_DiffTransformerEncoderLayer.

Sharding (8 cores, no collectives): core c handles batch b = c//4 and
query-block s = c%4 (256 query rows).  Each core computes K/V for its
whole batch (duplicated 4x inside a batch group - cheaper than a
cross-core exchange), its own Q rows, attention with the diff-MLP
score bias, and the residual/LN/FFN stack for its rows.

The diff MLP (Linear(1,32) -> ReLU -> Linear(32,1)) is a scalar
piecewise-linear function f(d) of d = |mz_i - mz_j| in [0,1).  Hidden
units whose ReLU knot -db1/dw1 falls outside (0,1) are linear or zero
on the whole domain and fold into a single alpha*d + beta term; only
the remaining knots (14 for the shipped weights) are evaluated, each
as one DVE tensor_scalar op via
    s*relu(a*d + b) = (d * s*a) max/min (-s*b)   [+ s*b folded into beta]
Accumulation of the terms runs on the PE as fp16 identity-matmul
accumulates into PSUM, which also spreads the result into every
head's score tile.

Matmuls run in fp16 (full PE rate, fp32 PSUM accumulate).  exp() is
computed with a constant -5 bias so its fp16 output cannot overflow;
the shift cancels in the softmax normalization.
"""
import numpy as np
from contextlib import ExitStack

B, L, DM, H, DK, FF = 2, 1024, 512, 8, 64, 2048
NCORES = 8
QB = 4                # query blocks per batch
LQ = L // QB          # 256 query rows per core
KT = L // 128         # 8 key tiles
TT = LQ // 128        # 2 token tiles per core
EPS = 1e-5
EXPB = -5.0           # constant exp bias (cancels in normalization)

_CACHE = {}
LAST_EXEC_NS = None


def _diff_consts(dw1, db1, dw2, db2):
    """Reduce the 32-unit scalar MLP over d in [0,1) to
    alpha*d + beta + sum_j s_j*relu(aa_j*d + bb_j) with in-domain knots."""
    safe = np.where(dw1 == 0, 1.0, dw1)
    t = np.where(dw1 != 0, -db1 / safe, np.inf)
    act = (t > 0) & (t < 1) & (dw1 != 0) & (dw2 != 0)
    on = (((dw1 > 0) & (t <= 0)) | ((dw1 < 0) & (t >= 1)) |
          ((dw1 == 0) & (db1 > 0)))
    db2 = float(np.asarray(db2).reshape(-1)[0]) if np.asarray(db2).size else 0.0
    alpha = float((dw2[on] * dw1[on]).sum())
    beta = float((dw2[on] * db1[on]).sum()) + db2
    s = np.sign(dw2[act])
    aa = np.abs(dw2[act]) * dw1[act]
    bb = np.abs(dw2[act]) * db1[act]
    beta += float((s * bb).sum())
    f0 = float((np.maximum(db1, 0) * dw2).sum()) + db2
    terms = tuple((float(x), float(y), float(z)) for x, y, z in zip(s, aa, bb))
    return alpha, beta, f0, terms


def _build(alpha, beta, terms):
    import concourse.bacc as bacc
    import concourse.tile as tile
    from concourse import mybir

    F32 = mybir.dt.float32
    F16 = mybir.dt.float16
    AT = mybir.ActivationFunctionType
    OP = mybir.AluOpType

    nc = bacc.Bacc("TRN2", target_bir_lowering=False, debug=False,
                   num_devices=NCORES)

    def din(name, shape, dt=F32):
        return nc.dram_tensor(name, shape, dt, kind="ExternalInput").ap()

    wq = din("wq", [DM, DM], F16);  wk = din("wk", [DM, DM], F16)
    wv = din("wv", [DM, DM], F16);  wo = din("wo", [DM, DM], F16)
    wf1 = din("wf1", [DM, FF], F16); wf2 = din("wf2", [FF, DM], F16)
    xb = din("xb", [L, DM], F16);   xq = din("xq", [LQ, DM])
    mzqb = din("mzqb", [128, LQ]); mzkc = din("mzkc", [128, KT])
    m01 = din("m01", [128, 1]);    c0t = din("c0t", [128, KT])
    ident = din("ident", [128, 128], F16)
    y = nc.dram_tensor("y", [LQ, DM], F32, kind="ExternalOutput").ap()

    with tile.TileContext(nc) as tc:
        with ExitStack() as ctx:
            body(ctx, tc, nc, mybir, F32, F16, AT, OP,
                 wq, wk, wv, wo, wf1, wf2, xb, xq, mzqb, mzkc, m01, c0t,
                 ident, y, alpha, beta, terms)
    nc.compile()
    return nc


def body(ctx, tc, nc, mybir, F32, F16, AT, OP,
         wq, wk, wv, wo, wf1, wf2, xb, xq, mzqb, mzkc, m01, c0t,
         ident, y, alpha, beta, terms):
    AF = KT * LQ         # 2048: diff / per-head score free size
    # ---------------- pools ----------------
    wpool = ctx.enter_context(tc.tile_pool(name="wpool", bufs=1))
    wbig = ctx.enter_context(tc.tile_pool(name="wbig", bufs=1))
    xpool = ctx.enter_context(tc.tile_pool(name="xpool", bufs=2))
    per = ctx.enter_context(tc.tile_pool(name="per", bufs=1))
    upool = ctx.enter_context(tc.tile_pool(name="upool", bufs=3))
    ptpool = ctx.enter_context(tc.tile_pool(name="ptpool", bufs=2))
    small = ctx.enter_context(tc.tile_pool(name="small", bufs=2))

    # ---------------- weight + input DMA ----------------
    def wload(name, src, kchunks, fdim):
        t = wpool.tile([128, kchunks * fdim], F16, name=name)
        nc.sync.dma_start(
            t[:].rearrange("p (kc f) -> p kc f", kc=kchunks),
            src.rearrange("(kc p) f -> p kc f", p=128))
        return t[:].rearrange("p (kc f) -> p kc f", kc=kchunks)

    wq_sb = wload("wq_sb", wq, 4, DM)
    wk_sb = wload("wk_sb", wk, 4, DM)
    wv_sb = wload("wv_sb", wv, 4, DM)
    wo_sb = wload("wo_sb", wo, 4, DM)

    id_sb = per.tile([128, 128], F16)
    nc.sync.dma_start(id_sb[:], ident)
    mzq_sb = per.tile([128, LQ], F32)
    nc.sync.dma_start(mzq_sb[:], mzqb)
    mzk_sb = per.tile([128, KT], F32)
    nc.sync.dma_start(mzk_sb[:], mzkc)
    m01_sb = per.tile([128, 1], F32)
    nc.sync.dma_start(m01_sb[:], m01)
    c0_sb = per.tile([128, KT], F32)
    nc.sync.dma_start(c0_sb[:], c0t)

    xq_sb = per.tile([128, TT * DM], F32, name="xq_sb").rearrange("p (t f) -> p t f", t=TT)
    for t in range(TT):
        nc.sync.dma_start(xq_sb[:, t, :], xq[t * 128:(t + 1) * 128, :])
    xqh = per.tile([128, TT * DM], F16, name="xqh").rearrange("p (t f) -> p t f", t=TT)
    for t in range(TT):
        nc.vector.tensor_copy(out=xqh[:, t, :], in_=xq_sb[:, t, :])

    # ---------------- transposes: xbT, xqT (fp16 in, fp32 psum out) -------
    pp = ctx.enter_context(tc.tile_pool(name="pp", bufs=4, space="PSUM"))
    ppA = ctx.enter_context(tc.tile_pool(name="ppA", bufs=4, space="PSUM"))

    xbT = per.tile([128, 4 * L], F16, name="xbT").rearrange("p (fc t) -> p fc t", fc=4)
    for fc in range(4):
        for g in range(2):          # two groups of 4 k-tiles
            tp = pp.tile([128, 512], F16, tag="bank", padded_shape=[128, 1024])
            for i in range(4):
                kt = g * 4 + i
                xt = xpool.tile([128, DM], F16, tag="xbt", name=f"xt{fc}_{kt}")
                nc.sync.dma_start(xt[:], xb[kt * 128:(kt + 1) * 128, :])
                nc.tensor.transpose(tp[:, i * 128:(i + 1) * 128],
                                    xt[:, fc * 128:(fc + 1) * 128], id_sb[:])
            nc.scalar.copy(out=xbT[:, fc, g * 512:(g + 1) * 512], in_=tp[:])

    xqT = per.tile([128, 4 * LQ], F16, name="xqT").rearrange("p (fc t) -> p fc t", fc=4)
    for fc in range(4):
        tp = pp.tile([128, 512], F16, tag="bank", padded_shape=[128, 1024])
        for t in range(TT):
            nc.tensor.transpose(tp[:, t * 128:(t + 1) * 128],
                                xqh[:, t, fc * 128:(fc + 1) * 128], id_sb[:])
        nc.scalar.copy(out=xqT[:, fc, :], in_=tp[:, 0:LQ])

    # ---------------- K/V/Q projections ----------------
    # kT: [feat, tok] feature-major keys
    kT = per.tile([128, 4 * L], F16, name="kT").rearrange("p (fc t) -> p fc t", fc=4)
    for fc in range(4):
        for g in range(2):
            kp = pp.tile([128, 512], F32, tag="bank")
            for kc in range(4):
                nc.tensor.matmul(
                    kp[:],
                    wk_sb[:, kc, fc * 128:(fc + 1) * 128],
                    xbT[:, kc, g * 512:(g + 1) * 512],
                    start=(kc == 0), stop=(kc == 3))
            nc.vector.tensor_copy(out=kT[:, fc, g * 512:(g + 1) * 512], in_=kp[:])

    # v: token-major, padded per-head with a ones column (denominator row)
    v_sb = []
    for kt in range(KT):
        vt = per.tile([128, H * (DK + 1)], F16, name=f"v_sb{kt}")
        v_sb.append(vt)
        vv = vt[:].rearrange("p (h f) -> p h f", h=H)
        nc.gpsimd.memset(vv[:, :, DK:DK + 1], 1.0)
        vp = pp.tile([128, 512], F32, tag="bank")
        for kc in range(4):
            nc.tensor.matmul(
                vp[:],
                xbT[:, kc, kt * 128:(kt + 1) * 128],
                wv_sb[:, kc, :],
                start=(kc == 0), stop=(kc == 3))
        nc.vector.tensor_copy(
            out=vv[:, :, 0:DK],
            in_=vp[:].rearrange("p (h f) -> p h f", h=H))

    # qT: [feat, tok] feature-major queries (wq pre-scaled by 1/sqrt(dk))
    qT = per.tile([128, 4 * LQ], F16, name="qT").rearrange("p (fc t) -> p fc t", fc=4)
    for fc in range(4):
        qp = pp.tile([128, 512], F32, tag="bank")
        for kc in range(4):
            nc.tensor.matmul(
                qp[:, 0:LQ],
                wq_sb[:, kc, fc * 128:(fc + 1) * 128],
                xqT[:, kc, :],
                start=(kc == 0), stop=(kc == 3))
        nc.scalar.copy(out=qT[:, fc, :], in_=qp[:, 0:LQ])

    # ---------------- diff-MLP score bias ----------------
    # D_all[p, kt*LQ + qi] = |mz_k - mz_q|,  acc = f(D) accumulated on PE

    D_all = per.tile([128, AF], F32)
    for kt in range(KT):
        nc.vector.tensor_scalar(
            out=D_all[:, kt * LQ:(kt + 1) * LQ], in0=mzq_sb[:],
            scalar1=mzk_sb[:, kt:kt + 1], scalar2=None, op0=OP.subtract)
    nc.gpsimd.memset(D_all[0:1, 0:LQ], 0.0)     # global-token row k=0
    nc.scalar.activation(out=D_all[:], in_=D_all[:], func=AT.Abs)

    acc_ps = [ppA.tile([128, 512], F32, tag="acc", name=f"acc_ps{n}")
              for n in range(AF // 512)]
    nterm = len(terms) + 1
    for j in range(nterm):
        u = upool.tile([128, AF], F16, tag="u", name=f"u{j}")
        if j == 0:
            nc.vector.tensor_scalar(
                out=u[:], in0=D_all[:], scalar1=float(alpha),
                scalar2=float(beta), op0=OP.mult, op1=OP.add)
        else:
            s, aa, bb = terms[j - 1]
            nc.vector.tensor_scalar(
                out=u[:], in0=D_all[:], scalar1=float(s * aa),
                scalar2=float(-s * bb), op0=OP.mult,
                op1=(OP.max if s > 0 else OP.min))
        for n in range(AF // 512):
            nc.tensor.matmul(
                acc_ps[n][:], id_sb[:],
                u[:, n * 512:(n + 1) * 512],
                start=(j == 0), stop=(j == nterm - 1))

    acc_sb = per.tile([128, AF], F16)
    for n in range(AF // 512):
        nc.scalar.copy(out=acc_sb[:, n * 512:(n + 1) * 512], in_=acc_ps[n][:])
    # global-token column q=0 (only on cores owning it): acc = acc*m01 + c0t
    accv = acc_sb[:].rearrange("p (kt q) -> p kt q", kt=KT)
    nc.vector.scalar_tensor_tensor(
        out=accv[:, :, 0], in0=accv[:, :, 0], scalar=m01_sb[:, 0:1],
        in1=c0_sb[:], op0=OP.mult, op1=OP.add)

    # ---------------- attention ----------------

    ones64 = per.tile([128, DK], F16)
    nc.gpsimd.memset(ones64[:], 1.0)
    expb_sb = per.tile([128, 1], F32)
    nc.gpsimd.memset(expb_sb[:], EXPB)
    eps_sb = per.tile([128, 1], F32)
    nc.gpsimd.memset(eps_sb[:], EPS)
    r_all = per.tile([128, 4 * 2 * LQ], F32)
    r_h = per.tile([128, 4 * 2 * LQ], F16)

    ctxT = per.tile([128, 4 * LQ], F16, name="ctxT").rearrange("p (hp t) -> p hp t", hp=4)
    stage = per.tile([128, 4 * LQ], F16, name="stage").rearrange("p (hh t) -> p hh t", hh=4)

    ctx_ps = {}
    for h in range(H):
        hp, lo = h // 2, h % 2
        # scores.T + diff spread, in two half-head psum tiles
        pt = ptpool.tile([128, AF], F16, tag="pt", name=f"pt{h}")
        for half in range(4):
            st = pp.tile([128, 512], F32, tag="bank", name=f"st{h}_{half}")
            for i in range(2):
                kt = half * 2 + i
                lhsT = kT[64 * lo:64 * lo + 64, hp, kt * 128:(kt + 1) * 128]
                rhs = qT[64 * lo:64 * lo + 64, hp, :]
                nc.tensor.matmul(st[:, i * LQ:(i + 1) * LQ], lhsT, rhs,
                                 start=True, stop=False)
                nc.tensor.matmul(st[:, i * LQ:(i + 1) * LQ], id_sb[:],
                                 acc_sb[:, kt * LQ:(kt + 1) * LQ],
                                 start=False, stop=True)
            nc.scalar.activation(out=pt[:, half * 512:(half + 1) * 512],
                                 in_=st[:], func=AT.Exp, bias=expb_sb[:])
        # ctx.T (+ denominator row 64) accumulated over k tiles
        if lo == 0:
            ctx_ps[hp] = pp.tile([DK + 1, 2 * LQ], F32, tag="bank",
                                 name=f"cx{hp}")
        cp = ctx_ps[hp]
        for kt in range(KT):
            nc.tensor.matmul(
                cp[:, lo * LQ:(lo + 1) * LQ],
                v_sb[kt][:].rearrange("p (h f) -> p h f", h=H)[:, h, :],
                pt[:, kt * LQ:(kt + 1) * LQ],
                start=(kt == 0), stop=(kt == KT - 1))
        if lo == 1:
            # denominators -> reciprocal -> fp16 -> broadcast -> normalize
            nc.vector.reciprocal_approx_fast(
                out=r_all[DK:DK + 1, hp * 512:(hp + 1) * 512],
                in_=cp[DK:DK + 1, :])
            nc.vector.tensor_copy(
                out=r_h[DK:DK + 1, hp * 512:(hp + 1) * 512],
                in_=r_all[DK:DK + 1, hp * 512:(hp + 1) * 512])
            for l2 in range(2):
                h2 = 2 * hp + l2
                rb = pp.tile([DK, LQ], F32, tag="bank", name=f"rb{h2}")
                nc.tensor.matmul(
                    rb[:], ones64[DK:DK + 1, :],
                    r_h[DK:DK + 1, hp * 512 + l2 * LQ: hp * 512 + (l2 + 1) * LQ],
                    start=True, stop=True)
                rbs = small.tile([DK, LQ], F32, tag="rbs", name=f"rbs{h2}")
                nc.scalar.copy(out=rbs[:], in_=rb[:])
                if l2 == 0:
                    nc.vector.scalar_tensor_tensor(
                        out=ctxT[0:DK, hp, :], in0=cp[0:DK, l2 * LQ:(l2 + 1) * LQ],
                        scalar=0.0, in1=rbs[:], op0=OP.bypass, op1=OP.mult)
                else:
                    nc.vector.scalar_tensor_tensor(
                        out=stage[0:DK, hp, :], in0=cp[0:DK, l2 * LQ:(l2 + 1) * LQ],
                        scalar=0.0, in1=rbs[:], op0=OP.bypass, op1=OP.mult)
                    nc.sync.dma_start(ctxT[DK:128, hp, :], stage[0:DK, hp, :])

    # ---------------- output projection + residual + LN1 ----------------
    x1 = per.tile([128, TT * DM], F32, name="x1").rearrange("p (t f) -> p t f", t=TT)
    xln = per.tile([128, TT * DM], F32, name="xln").rearrange("p (t f) -> p t f", t=TT)
    mv = small.tile([128, 2 * TT * 2], F32, tag="mv")

    def layernorm(src_ps, res_sb, out_sb, mvofs, tokens):
        # x1 = residual + psum; mean/var; out = (x1 - m) * rsqrt(v + eps)
        nc.vector.scalar_tensor_tensor(
            out=x1[:, tokens, :], in0=src_ps[:], scalar=0.0, in1=res_sb,
            op0=OP.bypass, op1=OP.add)
        st6 = small.tile([128, 6], F32, tag="st6")
        nc.vector.bn_stats(out=st6[:], in_=x1[:, tokens, :])
        m2 = mv[:, mvofs:mvofs + 2]
        nc.vector.bn_aggr(out=m2, in_=st6[:])
        nc.scalar.activation(out=m2[:, 1:2], in_=m2[:, 1:2], func=AT.Ln,
                             bias=eps_sb[:])
        nc.scalar.activation(out=m2[:, 1:2], in_=m2[:, 1:2], func=AT.Exp,
                             scale=-0.5)
        nc.vector.tensor_scalar(
            out=out_sb, in0=x1[:, tokens, :], scalar1=m2[:, 0:1],
            scalar2=m2[:, 1:2], op0=OP.subtract, op1=OP.mult)

    for t in range(TT):
        xp = pp.tile([128, DM], F32, tag="bank", name=f"xp{t}")
        for hp in range(4):
            nc.tensor.matmul(xp[:], ctxT[:, hp, t * 128:(t + 1) * 128],
                             wo_sb[:, hp, :], start=(hp == 0), stop=(hp == 3))
        layernorm(xp[:], xq_sb[:, t, :], xln[:, t, :], 4 * t, t)

    # ---------------- FFN ----------------
    wf1_sb = wbig.tile([128, 4 * FF], F16, tag="wf1")
    nc.sync.dma_start(
        wf1_sb[:].rearrange("p (kc f) -> p kc f", kc=4),
        wf1.rearrange("(kc p) f -> p kc f", p=128))
    wf1v = wf1_sb[:].rearrange("p (kc f) -> p kc f", kc=4)
    wf2_sb = wbig.tile([128, 16 * DM], F16, tag="wf2")
    nc.sync.dma_start(
        wf2_sb[:].rearrange("p (kc f) -> p kc f", kc=16),
        wf2.rearrange("(kc p) f -> p kc f", p=128))
    wf2v = wf2_sb[:].rearrange("p (kc f) -> p kc f", kc=16)

    # transpose LN1 output to feature-major fp16 for the FFN lhsT
    xlnh = per.tile([128, TT * DM], F16, name="xlnh").rearrange("p (t f) -> p t f", t=TT)
    for t in range(TT):
        nc.vector.tensor_copy(out=xlnh[:, t, :], in_=xln[:, t, :])
    xlnT = per.tile([128, 4 * LQ], F16, name="xlnT").rearrange("p (fc t) -> p fc t", fc=4)
    for fc in range(4):
        tp = pp.tile([128, 512], F16, tag="bank", padded_shape=[128, 1024])
        for t in range(TT):
            nc.tensor.transpose(tp[:, t * 128:(t + 1) * 128],
                                xlnh[:, t, fc * 128:(fc + 1) * 128], id_sb[:])
        nc.scalar.copy(out=xlnT[:, fc, :], in_=tp[:, 0:LQ])

    # FFN1: hid-major relu'd activations, 16 M-chunks of 128
    f1r = per.tile([128, 16 * LQ], F16, name="f1r").rearrange("p (mc t) -> p mc t", mc=16)
    for g in range(8):
        fp = pp.tile([128, 512], F32, tag="bank", name=f"fp{g}")
        for i in range(2):
            mc = 2 * g + i
            for kc in range(4):
                nc.tensor.matmul(
                    fp[:, i * LQ:(i + 1) * LQ],
                    wf1v[:, kc, mc * 128:(mc + 1) * 128],
                    xlnT[:, kc, :],
                    start=(kc == 0), stop=(kc == 3))
        dst = f1r[:, 2 * g:2 * g + 2, :].rearrange("p a b -> p (a b)")
        if g % 2 == 0:
            nc.scalar.activation(out=dst, in_=fp[:], func=AT.Relu)
        else:
            nc.vector.tensor_scalar(out=dst, in0=fp[:], scalar1=0.0,
                                    scalar2=None, op0=OP.max)

    # FFN2 + residual + LN2 + store
    yout = per.tile([128, TT * DM], F32, name="yout").rearrange("p (t f) -> p t f", t=TT)
    for t in range(TT):
        f2 = pp.tile([128, DM], F32, tag="bank", name=f"f2{t}")
        for kc in range(16):
            nc.tensor.matmul(
                f2[:], f1r[:, kc, t * 128:(t + 1) * 128],
                wf2v[:, kc, :], start=(kc == 0), stop=(kc == 15))
        layernorm(f2[:], xln[:, t, :], yout[:, t, :], 4 * t + 2, t)
        nc.sync.dma_start(y[t * 128:(t + 1) * 128, :], yout[:, t, :])


def kernel(**inputs):
    global LAST_EXEC_NS
    from concourse.bass_utils import run_bass_kernel_spmd

    inp = {k: np.ascontiguousarray(np.asarray(v)) for k, v in inputs.items()}
    x = inp["x"].astype(np.float32)
    mz = inp["mz"].astype(np.float32)

    for k in ("bq", "bk", "bv", "bo", "bf1", "bf2", "b1", "b2"):
        assert not inp[k].any(), f"nonzero bias {k} unsupported"
    assert (inp["g1"] == 1).all() and (inp["g2"] == 1).all()
    assert not inp["pad_mask"].any()

    alpha, beta, f0, terms = _diff_consts(
        inp["dw1"].astype(np.float64), inp["db1"].astype(np.float64),
        inp["dw2"].astype(np.float64), inp["db2"].astype(np.float64))

    key = (alpha, beta, terms)
    if key not in _CACHE:
        _CACHE[key] = _build(alpha, beta, terms)
    nc = _CACHE[key]

    wq = (inp["Wq"].astype(np.float64) / np.sqrt(DK)).astype(np.float16)
    ident = np.eye(128, dtype=np.float16)

    in_maps = []
    for c in range(NCORES):
        b, s = c // 4, c % 4
        qr = slice(s * LQ, (s + 1) * LQ)
        mzb = mz[b, :, 0]
        own0 = (s == 0)
        m01 = np.full((128, 1), 0.0 if own0 else 1.0, np.float32)
        c0t = np.full((128, KT), f0 if own0 else 0.0, np.float32)
        in_maps.append({
            "wq": wq, "wk": inp["Wk"].astype(np.float16),
            "wv": inp["Wv"].astype(np.float16),
            "wo": inp["Wo"].astype(np.float16),
            "wf1": inp["Wf1"].astype(np.float16),
            "wf2": inp["Wf2"].astype(np.float16),
            "xb": x[b].astype(np.float16),
            "xq": np.ascontiguousarray(x[b, qr]),
            "mzqb": np.ascontiguousarray(
                np.broadcast_to(mzb[qr][None, :], (128, LQ))),
            "mzkc": np.ascontiguousarray(mzb.reshape(KT, 128).T),
            "m01": m01, "c0t": c0t, "ident": ident,
        })

    res = run_bass_kernel_spmd(nc, in_maps, list(range(NCORES)))
    if res.exec_time_ns is not None:
        LAST_EXEC_NS = res.exec_time_ns
    out = np.empty((B, L, DM), np.float32)
    for c in range(NCORES):
        b, s = c // 4, c % 4
        out[b, s * LQ:(s + 1) * LQ] = res.results[c]["y"]
    return out
